# revision 1
# baseline (speedup 1.0000x reference)
"""GroupedQueryAttention TRN2 kernel.

Sharding: 4-way tensor-parallel over heads x 2-way data-parallel over batch.
Core c handles batch b=c//4 and head-group gc=c%4 (kv heads {2gc, 2gc+1},
q heads (hkv, g) for g in 0..3 -> 8 q heads per core).

Per-core device program (T=2048, C=2048, D=64):
  Q^T = Wq_g @ x^T          [512, T]  (slot layout: partition p<64 -> kv0, p>=64 -> kv1)
  K^T = Wk_g @ x^T          [128, T]
  V^T = Wv_g @ x^T -> PE-transpose -> V [T, 130] with ones columns (rowsum trick)
  S^T tiles = K_tile Q^T (causal-restricted widths), exp on ScalarE,
  triangle mask on GpSimd, O'^T[65, t] accumulated on PE (row 64 = rowsum),
  normalize via PE broadcast of rowsum + DVE reciprocal/multiply,
  y_partial = O^T.T @ Wo_g  -> host sums 4 partials per batch.

All matmuls run in float32r (TF32-like, 1 cyc/row at moving dim >= 256);
memory stays fp32, APs are bitcast to float32r at the matmul.
"""

import numpy as np

import concourse.bass as bass
import concourse.mybir as mybir
import concourse.tile as tile
from concourse import bacc
from concourse.bass_utils import run_bass_kernel_spmd

H, HKV, D, G = 32, 8, 64, 4
B, T, C = 2, 2048, 2048
P = 128
NCORES = 8
F32 = mybir.dt.float32
FR = mybir.dt.float32r

NT = T // 512   # 4 q blocks of 512
NK = C // P     # 16 contraction tiles
NTT = T // P    # 16 key/row tiles

_NC_CACHE = None


def _fr(ap):
    return ap.bitcast(FR)


def build_kernel(nc, tc, ins, outs):
    xT, wqT, wkT, wvT, woR, tri, iden = (
        ins["xT"], ins["wqT"], ins["wkT"], ins["wvT"], ins["woR"], ins["tri"],
        ins["iden"],
    )
    y = outs["y"]
    EXP = mybir.ActivationFunctionType.Exp
    CPY = mybir.ActivationFunctionType.Copy

    # ---- persistent SBUF (one long-lived pool, distinct tags) ----
    persist = tc._persist_pool
    qT_sb = persist.tile([P, G, T], FR, name="qT_sb", tag="qT_sb")      # 4 MB
    kT_sb = persist.tile([P, T], FR, name="kT_sb", tag="kT_sb")         # 1 MB
    v_sb = persist.tile([P, NTT, 130], FR, name="v_sb", tag="v_sb")     # ~1 MB
    oT_sb = persist.tile([P, G, T], FR, name="oT_sb", tag="oT_sb")      # 4 MB
    wo_sb = persist.tile([P, G, C], FR, name="wo_sb", tag="wo_sb")      # 4 MB
    tri_sb = persist.tile([P, P], FR, name="tri_sb", tag="tri_sb")
    iden_sb = persist.tile([P, P], FR, name="iden_sb", tag="iden_sb")
    ones_sb = persist.tile([1, 64], FR, name="ones_sb", tag="ones_sb")

    nc.sync.dma_start(tri_sb[:], tri[:])
    nc.sync.dma_start(iden_sb[:], iden[:])
    nc.sync.dma_start(ones_sb[:], ins["ones64"][:])
    # ones columns of V' (64 and 129 per key tile); data columns overwritten below
    nc.sync.dma_start(v_sb[:], ins["vinit"][:])

    # ================= phase A: projections =================
    # k-outer loop: 6 PSUM accumulators (4 Q slots + K + V) share each x tile,
    # so only a few x tiles are live at a time.
    with (
        tc.tile_pool(name="wproj", bufs=1) as wpool,
        tc.tile_pool(name="xt", bufs=6) as xpool,
        tc.tile_pool(name="vtt", bufs=1) as vttpool,
        tc.tile_pool(name="pp", bufs=6, space="PSUM") as pp,
    ):
        wq_sb = wpool.tile([P, NK, 512], FR, name="wq_sb")   # 4 MB
        wk_sb = wpool.tile([P, NK, P], FR, name="wk_sb")
        wv_sb = wpool.tile([P, NK, P], FR, name="wv_sb")
        vTt = vttpool.tile([P, T], FR, name="vTt")           # V^T temp, 1 MB
        nc.sync.dma_start(wq_sb[:], wqT.rearrange("(ko p) m -> p ko m", p=P))
        nc.sync.dma_start(wk_sb[:], wkT.rearrange("(ko p) m -> p ko m", p=P))
        nc.sync.dma_start(wv_sb[:], wvT.rearrange("(ko p) m -> p ko m", p=P))

        for tb in range(NT):
            ts = slice(tb * 512, (tb + 1) * 512)
            psq = [pp.tile([P, 512], F32, tag="pp", name=f"psq_{tb}_{g}")
                   for g in range(G)]
            psk = pp.tile([P, 512], F32, tag="pp", name=f"psk_{tb}")
            psv = pp.tile([P, 512], F32, tag="pp", name=f"psv_{tb}")
            for k in range(NK):
                xt = xpool.tile([P, 512], FR, tag="xt", name=f"xt_{tb}_{k}")
                nc.sync.dma_start(xt[:], xT[k * P:(k + 1) * P, ts])
                st, sp = (k == 0), (k == NK - 1)
                for g in range(G):
                    nc.tensor.matmul(psq[g][:],
                                     wq_sb[:, k, g * P:(g + 1) * P],
                                     xt[:], start=st, stop=sp)
                nc.tensor.matmul(psk[:], wk_sb[:, k, :], xt[:],
                                 start=st, stop=sp)
                nc.tensor.matmul(psv[:], wv_sb[:, k, :], xt[:],
                                 start=st, stop=sp)
            for g in range(G):
                nc.scalar.activation(qT_sb[:, g, ts], psq[g][:], CPY)
            nc.scalar.activation(kT_sb[:, ts], psk[:], CPY)
            nc.scalar.activation(vTt[:, ts], psv[:], CPY)

        # V^T -> V (PE transpose per 128x128 tile), into V' layout with ones cols
        with tc.tile_pool(name="pvt", bufs=2, space="PSUM") as pvt:
            for tt in range(NTT):
                pt_ = pvt.tile([P, P], FR, tag="pvt", name=f"pvt_{tt}")
                nc.tensor.transpose(pt_[:], vTt[:, tt * P:(tt + 1) * P],
                                    iden_sb[:])
                nc.vector.tensor_copy(v_sb[:, tt, 0:64], pt_[:, 0:64])
                nc.vector.tensor_copy(v_sb[:, tt, 65:129], pt_[:, 64:128])

    # ================= phase B: attention =================
    nc.sync.dma_start(wo_sb[:], woR.rearrange("(m p) c -> p m c", p=P))
    with (
        tc.tile_pool(name="ps", bufs=3, space="PSUM") as pspool,
        tc.tile_pool(name="po", bufs=2, space="PSUM") as popool,
        tc.tile_pool(name="ptp", bufs=6) as ptpool,
        tc.tile_pool(name="rs", bufs=2) as rspool,
        tc.tile_pool(name="rb", bufs=2) as rbpool,
    ):
        for j in range(NT):
            qs0 = j * 512
            for g in range(G):
                po = [
                    popool.tile([P, 512], F32, tag="po", name=f"po_{j}_{g}_{h}")
                    for h in range(2)
                ]
                ilast = 4 * j + 3
                for i in range(4 * j + 4):
                    loc = max(0, P * i - qs0)
                    ps = pspool.tile([P, 2, 512], F32, tag="ps",
                                     name=f"ps_{j}_{g}_{i}")
                    pt_ = ptpool.tile([P, 2, 512], FR, tag="pt",
                                      name=f"pt_{j}_{g}_{i}")
                    for h in range(2):
                        nc.tensor.matmul(
                            ps[:, h, loc:512],
                            kT_sb[h * 64:(h + 1) * 64, i * P:(i + 1) * P],
                            qT_sb[h * 64:(h + 1) * 64, g, qs0 + loc:qs0 + 512],
                            start=True, stop=True,
                        )
                    nc.scalar.activation(pt_[:, :, loc:512], ps[:, :, loc:512],
                                         EXP, scale=0.125)
                    if i >= 4 * j:  # diagonal tile: mask strict-lower triangle
                        nc.gpsimd.tensor_mul(
                            pt_[:, :, loc:loc + P], pt_[:, :, loc:loc + P],
                            tri_sb[:, None, :].to_broadcast([P, 2, P]),
                        )
                    for h in range(2):
                        nc.tensor.matmul(
                            po[h][0:65, loc:512],
                            v_sb[:, i, h * 65:h * 65 + 65],
                            pt_[:, h, loc:512],
                            start=(i == 0), stop=(i == ilast),
                        )
                for h in range(2):
                    rs = rspool.tile([1, 512], FR, tag="rs", name=f"rs_{j}_{g}_{h}")
                    nc.scalar.activation(rs[:], po[h][64:65, :], CPY)
                    # broadcast rowsum via a slot borrowed from the ps pool
                    pru = pspool.tile([P, 2, 512], F32, tag="ps",
                                      name=f"pr_{j}_{g}_{h}")
                    nc.tensor.matmul(pru[0:64, 0, :], ones_sb[:], rs[:],
                                     start=True, stop=True)
                    rb = rbpool.tile([64, 512], F32, tag="rb", name=f"rb_{j}_{g}_{h}")
                    nc.vector.reciprocal(rb[:], pru[0:64, 0, :])
                    nc.vector.tensor_mul(
                        oT_sb[h * 64:(h + 1) * 64, g, qs0:qs0 + 512],
                        po[h][0:64, :], rb[:],
                    )

    # ================= phase C: output projection =================
    with (
        tc.tile_pool(name="py", bufs=4, space="PSUM") as pypool,
        tc.tile_pool(name="ysb", bufs=4) as ypool,
    ):
        for tt in range(NTT):
            for cb in range(NT):
                py = pypool.tile([P, 512], F32, tag="py", name=f"py_{tt}_{cb}")
                for m in range(G):
                    nc.tensor.matmul(
                        py[:], oT_sb[:, m, tt * P:(tt + 1) * P],
                        wo_sb[:, m, cb * 512:(cb + 1) * 512],
                        start=(m == 0), stop=(m == G - 1),
                    )
                ysb = ypool.tile([P, 512], F32, tag="ysb", name=f"y_{tt}_{cb}")
                nc.vector.tensor_copy(ysb[:], py[:])
                nc.sync.dma_start(y[tt * P:(tt + 1) * P, cb * 512:(cb + 1) * 512],
                                  ysb[:])


def build_nc():
    global _NC_CACHE
    if _NC_CACHE is not None:
        return _NC_CACHE
    nc = bacc.Bacc("TRN2", debug=False, target_bir_lowering=False,
                   num_devices=NCORES)
    ins = {
        "xT": nc.dram_tensor("xT", [C, T], FR, kind="ExternalInput").ap(),
        "wqT": nc.dram_tensor("wqT", [C, 512], FR, kind="ExternalInput").ap(),
        "wkT": nc.dram_tensor("wkT", [C, P], FR, kind="ExternalInput").ap(),
        "wvT": nc.dram_tensor("wvT", [C, P], FR, kind="ExternalInput").ap(),
        "woR": nc.dram_tensor("woR", [512, C], FR, kind="ExternalInput").ap(),
        "tri": nc.dram_tensor("tri", [P, P], FR, kind="ExternalInput").ap(),
        "iden": nc.dram_tensor("iden", [P, P], FR, kind="ExternalInput").ap(),
        "ones64": nc.dram_tensor("ones64", [1, 64], FR, kind="ExternalInput").ap(),
        "vinit": nc.dram_tensor("vinit", [P, NTT, 130], FR,
                                kind="ExternalInput").ap(),
    }
    outs = {"y": nc.dram_tensor("y", [T, C], F32, kind="ExternalOutput").ap()}
    with tile.TileContext(nc) as tc:
        with tc.tile_pool(name="persist", bufs=1) as persist:
            tc._persist_pool = persist
            build_kernel(nc, tc, ins, outs)
    nc.compile()
    _NC_CACHE = nc
    return nc


def make_core_inputs(x, Wq, Wkv, Wo):
    """Host-side shard + pre-transpose. Returns list of 8 in_maps."""
    x = np.asarray(x, np.float32)
    Wq = np.asarray(Wq, np.float32)
    Wkv = np.asarray(Wkv, np.float32)
    Wo = np.asarray(Wo, np.float32)
    tri = np.triu(np.ones((P, P), np.float32))   # keep t_local >= s_local
    iden = np.eye(P, dtype=np.float32)
    in_maps = []
    for c in range(NCORES):
        b, gc = c // 4, c % 4
        xT = np.ascontiguousarray(x[b].T)                       # [C, T]
        Wq4 = Wq.reshape(HKV, G, D, C)[2 * gc:2 * gc + 2]       # [2, G, D, C]
        wqT = np.ascontiguousarray(
            np.transpose(Wq4, (1, 0, 2, 3)).reshape(512, C).T)  # [C, (g,kv,d)]
        wkT = np.ascontiguousarray(Wkv[2 * gc * 64:(2 * gc + 2) * 64].T)
        wvT = np.ascontiguousarray(
            Wkv[HKV * D + 2 * gc * 64:HKV * D + (2 * gc + 2) * 64].T)
        Wo4 = Wo.reshape(C, HKV, G, D)[:, 2 * gc:2 * gc + 2]    # [C, 2, G, D]
        woR = np.ascontiguousarray(
            np.transpose(Wo4, (2, 1, 3, 0)).reshape(512, C))    # [(g,kv,d), C]
        in_maps.append({"xT": xT, "wqT": wqT, "wkT": wkT, "wvT": wvT,
                        "woR": woR, "tri": tri, "iden": iden,
                        "ones64": np.ones((1, 64), np.float32),
                        "vinit": np.ones((P, NTT, 130), np.float32)})
    return in_maps


def kernel(x, Wq, Wkv, Wo, trace=False):
    nc = build_nc()
    in_maps = make_core_inputs(x, Wq, Wkv, Wo)
    res = run_bass_kernel_spmd(nc, in_maps, core_ids=list(range(NCORES)),
                               trace=trace)
    y = np.zeros((B, T, C), np.float32)
    for c in range(NCORES):
        y[c // 4] += res.results[c]["y"]
    if trace:
        kernel.last_exec_time_ns = res.exec_time_ns
        kernel.last_results = res
    return y



# revision 5
# speedup vs baseline: 1.4397x; 1.4397x over previous
"""GroupedQueryAttention TRN2 kernel (bf16).

Sharding: 4-way tensor-parallel over heads x 2-way data-parallel over batch.
Core c handles batch b=c//4 and head-group gc=c%4 (kv heads {2gc, 2gc+1},
q heads (hkv, g) for g in 0..3 -> 8 q heads per core).

Per-core device program (T=2048, C=2048, D=64), all matmul operands bf16
(PSUM accumulation fp32):
  phase A (PE-bound ~82us): Q^T/K^T/V^T projections per 512-col block,
    V PE-transposed into V' [T,130] with ones cols (rowsum trick).
  phase B (Scalar exp-bound): S^T tiles = K_tile Q^T (causal widths),
    exp on ScalarE -> bf16, diag mask on GpSimd, O'^T accumulated on PE
    (row 64 = rowsum), normalize via ones2-matmul broadcast of rowsums +
    DVE reciprocal_approx_fast + DVE multiply.
  phase C (PE-bound): y_tile = O^T.T @ Wo_g, interleaved INTO phase B at
    (j,g) boundaries so PE fills exp-wait stalls with output-proj matmuls.
  Host sums the 4 TP partials per batch in fp32.

PSUM budget in B||C: ps 2x2 banks + po 2x1 + py/pru 2x1 = 8 banks.
"""

import numpy as np
import ml_dtypes

import concourse.bass as bass
import concourse.mybir as mybir
import concourse.tile as tile
from concourse import bacc
from concourse.bass_utils import run_bass_kernel_spmd

H, HKV, D, G = 32, 8, 64, 4
B, T, C = 2, 2048, 2048
P = 128
NCORES = 8
F32 = mybir.dt.float32
BF = mybir.dt.bfloat16
BFNP = ml_dtypes.bfloat16

NT = T // 512   # 4 q blocks of 512
NK = C // P     # 16 contraction tiles
NTT = T // P    # 16 key/row tiles

_NC_CACHE = None


def build_kernel(nc, tc, ins, outs):
    xTr = ins["xT"].rearrange("(ko p) t -> p ko t", p=P)
    y = outs["y"]
    EXP = mybir.ActivationFunctionType.Exp
    CPY = mybir.ActivationFunctionType.Copy

    # ---- persistent SBUF ----
    persist = tc._persist_pool
    qT_sb = persist.tile([P, G, T], BF, name="qT_sb", tag="qT_sb")
    kT_sb = persist.tile([P, T], BF, name="kT_sb", tag="kT_sb")
    v_sb = persist.tile([P, NTT, 130], BF, name="v_sb", tag="v_sb")
    oT_sb = persist.tile([P, G, T], BF, name="oT_sb", tag="oT_sb")
    wo_sb = persist.tile([P, G, C], BF, name="wo_sb", tag="wo_sb")
    tri_sb = persist.tile([P, P], BF, name="tri_sb", tag="tri_sb")
    iden_sb = persist.tile([P, P], BF, name="iden_sb", tag="iden_sb")
    ones2_sb = persist.tile([2, P], BF, name="ones2_sb", tag="ones2_sb")

    nc.sync.dma_start(tri_sb[:], ins["tri"][:])
    nc.sync.dma_start(iden_sb[:], ins["iden"][:])
    nc.sync.dma_start(ones2_sb[:], ins["ones2"][:])
    # ones columns of V' (64 and 129 per key tile); data columns overwritten
    nc.sync.dma_start(v_sb[:], ins["vinit"][:])

    # ================= phase A: projections =================
    with (
        tc.tile_pool(name="wproj", bufs=1) as wpool,
        tc.tile_pool(name="xt", bufs=2) as xpool,
        tc.tile_pool(name="vtt", bufs=2) as vttpool,
        tc.tile_pool(name="pp", bufs=6, space="PSUM") as pp,
        tc.tile_pool(name="pvt", bufs=2, space="PSUM") as pvt,
    ):
        wq_sb = wpool.tile([P, NK, 512], BF, name="wq_sb")
        wk_sb = wpool.tile([P, NK, P], BF, name="wk_sb")
        wv_sb = wpool.tile([P, NK, P], BF, name="wv_sb")
        # chunked weight DMA so the first matmul doesn't wait on 2 MB
        for c4 in range(4):
            nc.sync.dma_start(
                wq_sb[:, 4 * c4:4 * c4 + 4, :],
                ins["wqT"][512 * c4:512 * (c4 + 1), :].rearrange(
                    "(ko p) m -> p ko m", p=P))
        nc.sync.dma_start(wk_sb[:], ins["wkT"].rearrange("(ko p) m -> p ko m", p=P))
        nc.sync.dma_start(wv_sb[:], ins["wvT"].rearrange("(ko p) m -> p ko m", p=P))

        for tb in range(NT):
            ts = slice(tb * 512, (tb + 1) * 512)
            xt = xpool.tile([P, NK, 512], BF, tag="xt", name=f"xt_{tb}")
            nc.sync.dma_start(xt[:], xTr[:, :, ts])
            psq = [pp.tile([P, 512], F32, tag="pp", name=f"psq_{tb}_{g}")
                   for g in range(G)]
            psk = pp.tile([P, 512], F32, tag="pp", name=f"psk_{tb}")
            psv = pp.tile([P, 512], F32, tag="pp", name=f"psv_{tb}")
            for k in range(NK):
                st, sp = (k == 0), (k == NK - 1)
                for g in range(G):
                    nc.tensor.matmul(psq[g][:],
                                     wq_sb[:, k, g * P:(g + 1) * P],
                                     xt[:, k, :], start=st, stop=sp)
                nc.tensor.matmul(psk[:], wk_sb[:, k, :], xt[:, k, :],
                                 start=st, stop=sp)
                nc.tensor.matmul(psv[:], wv_sb[:, k, :], xt[:, k, :],
                                 start=st, stop=sp)
            vtt = vttpool.tile([P, 512], BF, tag="vtt", name=f"vtt_{tb}")
            # copies split Scalar/DVE to shorten the per-tb tail
            nc.scalar.activation(qT_sb[:, 0, ts], psq[0][:], CPY)
            nc.scalar.activation(qT_sb[:, 1, ts], psq[1][:], CPY)
            nc.scalar.activation(kT_sb[:, ts], psk[:], CPY)
            nc.vector.tensor_copy(qT_sb[:, 2, ts], psq[2][:])
            nc.vector.tensor_copy(qT_sb[:, 3, ts], psq[3][:])
            nc.vector.tensor_copy(vtt[:], psv[:])
            # V^T -> V (PE transpose) into V' layout with ones cols
            for dd in range(4):
                tt = 4 * tb + dd
                pt_ = pvt.tile([P, P], BF, tag="pvt", name=f"pvt_{tt}")
                nc.tensor.transpose(pt_[:], vtt[:, dd * P:(dd + 1) * P],
                                    iden_sb[:])
                nc.vector.tensor_copy(v_sb[:, tt, 0:64], pt_[:, 0:64])
                nc.vector.tensor_copy(v_sb[:, tt, 65:129], pt_[:, 64:128])

    # ================= phase B || C =================
    for c4 in range(4):
        nc.sync.dma_start(
            wo_sb[:, c4, :],
            ins["woR"][128 * c4:128 * (c4 + 1), :].rearrange(
                "(m p) c -> p m c", p=P))

    with (
        tc.tile_pool(name="ps", bufs=2, space="PSUM") as pspool,
        tc.tile_pool(name="po", bufs=2, space="PSUM") as popool,
        tc.tile_pool(name="py", bufs=2, space="PSUM") as pypool,
        tc.tile_pool(name="ptp", bufs=6) as ptpool,
        tc.tile_pool(name="rs", bufs=4) as rspool,
        tc.tile_pool(name="rb", bufs=2) as rbpool,
        tc.tile_pool(name="ysb", bufs=2) as ypool,
    ):
        def emit_B_group(j, g):
            qs0 = j * 512
            po = [popool.tile([P, 512], F32, tag="po", name=f"po_{j}_{g}_{h}")
                  for h in range(2)]
            ilast = 4 * j + 3
            for i in range(4 * j + 4):
                loc = max(0, P * i - qs0)
                ps = pspool.tile([P, 2, 512], F32, tag="ps",
                                 name=f"ps_{j}_{g}_{i}")
                pt_ = ptpool.tile([P, 2, 512], BF, tag="pt",
                                  name=f"pt_{j}_{g}_{i}")
                for h in range(2):
                    nc.tensor.matmul(
                        ps[:, h, loc:512],
                        kT_sb[h * 64:(h + 1) * 64, i * P:(i + 1) * P],
                        qT_sb[h * 64:(h + 1) * 64, g, qs0 + loc:qs0 + 512],
                        start=True, stop=True,
                    )
                nc.scalar.activation(pt_[:, :, loc:512], ps[:, :, loc:512],
                                     EXP, scale=0.125)
                if i >= 4 * j:  # diagonal tile: mask strict-lower triangle
                    nc.gpsimd.tensor_mul(
                        pt_[:, :, loc:loc + P], pt_[:, :, loc:loc + P],
                        tri_sb[:, None, :].to_broadcast([P, 2, P]),
                    )
                for h in range(2):
                    nc.tensor.matmul(
                        po[h][0:65, loc:512],
                        v_sb[:, i, h * 65:h * 65 + 65],
                        pt_[:, h, loc:512],
                        start=(i == 0), stop=(i == ilast),
                    )
            # normalize: rowsums -> broadcast (ones matmul) -> 1/x -> mul
            rs = [rspool.tile([1, 512], BF, tag="rs", name=f"rs_{j}_{g}_{h}")
                  for h in range(2)]
            pru = pypool.tile([P, 512], F32, tag="py", name=f"pru_{j}_{g}")
            for h in range(2):
                nc.vector.tensor_copy(rs[h][:], po[h][64:65, :])
                nc.tensor.matmul(pru[h * 64:(h + 1) * 64, :], ones2_sb[0:1, 0:64],
                                 rs[h][:], start=True, stop=True)
            rb = rbpool.tile([P, 512], F32, tag="rb", name=f"rb_{j}_{g}")
            nc.vector.reciprocal_approx_fast(rb[:], pru[:])
            for h in range(2):
                nc.vector.tensor_mul(
                    oT_sb[h * 64:(h + 1) * 64, g, qs0:qs0 + 512],
                    po[h][0:64, :], rb[h * 64:(h + 1) * 64, :],
                )

        def emit_C_tt(tt):
            ysb = ypool.tile([P, T], BF, tag="ysb", name=f"y_{tt}")
            for cb in range(NT):
                py = pypool.tile([P, 512], F32, tag="py", name=f"py_{tt}_{cb}")
                for m in range(G):
                    nc.tensor.matmul(
                        py[:], oT_sb[:, m, tt * P:(tt + 1) * P],
                        wo_sb[:, m, cb * 512:(cb + 1) * 512],
                        start=(m == 0), stop=(m == G - 1),
                    )
                nc.vector.tensor_copy(ysb[:, cb * 512:(cb + 1) * 512], py[:])
            nc.sync.dma_start(y[tt * P:(tt + 1) * P, :], ysb[:])

        for j in range(NT):
            for g in range(G):
                emit_B_group(j, g)
                if j >= 1:
                    emit_C_tt(4 * (j - 1) + g)
        for dd in range(4):
            emit_C_tt(12 + dd)


def build_nc():
    global _NC_CACHE
    if _NC_CACHE is not None:
        return _NC_CACHE
    nc = bacc.Bacc("TRN2", debug=False, target_bir_lowering=False,
                   num_devices=NCORES)
    ins = {
        "xT": nc.dram_tensor("xT", [C, T], BF, kind="ExternalInput").ap(),
        "wqT": nc.dram_tensor("wqT", [C, 512], BF, kind="ExternalInput").ap(),
        "wkT": nc.dram_tensor("wkT", [C, P], BF, kind="ExternalInput").ap(),
        "wvT": nc.dram_tensor("wvT", [C, P], BF, kind="ExternalInput").ap(),
        "woR": nc.dram_tensor("woR", [512, C], BF, kind="ExternalInput").ap(),
        "tri": nc.dram_tensor("tri", [P, P], BF, kind="ExternalInput").ap(),
        "iden": nc.dram_tensor("iden", [P, P], BF, kind="ExternalInput").ap(),
        "ones2": nc.dram_tensor("ones2", [2, P], BF, kind="ExternalInput").ap(),
        "vinit": nc.dram_tensor("vinit", [P, NTT, 130], BF,
                                kind="ExternalInput").ap(),
    }
    outs = {"y": nc.dram_tensor("y", [T, C], BF, kind="ExternalOutput").ap()}
    with tile.TileContext(nc) as tc:
        with tc.tile_pool(name="persist", bufs=1) as persist:
            tc._persist_pool = persist
            build_kernel(nc, tc, ins, outs)
    nc.compile()
    _NC_CACHE = nc
    return nc


def make_core_inputs(x, Wq, Wkv, Wo):
    """Host-side shard + pre-transpose + bf16 cast. Returns 8 in_maps."""
    x = np.asarray(x, np.float32)
    Wq = np.asarray(Wq, np.float32)
    Wkv = np.asarray(Wkv, np.float32)
    Wo = np.asarray(Wo, np.float32)
    tri = np.triu(np.ones((P, P), np.float32))   # keep t_local >= s_local
    iden = np.eye(P, dtype=np.float32)
    # ones2: partitions 0-63 of the broadcast get rowsum h0, 64-127 get h1
    ones2 = np.zeros((2, P), np.float32)
    ones2[0, 0:64] = 1.0
    ones2[1, 64:128] = 1.0
    in_maps = []
    for c in range(NCORES):
        b, gc = c // 4, c % 4
        xT = np.ascontiguousarray(x[b].T)                       # [C, T]
        Wq4 = Wq.reshape(HKV, G, D, C)[2 * gc:2 * gc + 2]       # [2, G, D, C]
        wqT = np.ascontiguousarray(
            np.transpose(Wq4, (1, 0, 2, 3)).reshape(512, C).T)  # [C, (g,kv,d)]
        wkT = np.ascontiguousarray(Wkv[2 * gc * 64:(2 * gc + 2) * 64].T)
        wvT = np.ascontiguousarray(
            Wkv[HKV * D + 2 * gc * 64:HKV * D + (2 * gc + 2) * 64].T)
        Wo4 = Wo.reshape(C, HKV, G, D)[:, 2 * gc:2 * gc + 2]    # [C, 2, G, D]
        woR = np.ascontiguousarray(
            np.transpose(Wo4, (2, 1, 3, 0)).reshape(512, C))    # [(g,kv,d), C]
        in_maps.append({
            "xT": xT.astype(BFNP), "wqT": wqT.astype(BFNP),
            "wkT": wkT.astype(BFNP), "wvT": wvT.astype(BFNP),
            "woR": woR.astype(BFNP), "tri": tri.astype(BFNP),
            "iden": iden.astype(BFNP), "ones2": ones2.astype(BFNP),
            "vinit": np.ones((P, NTT, 130), BFNP),
        })
    return in_maps


def kernel(x, Wq, Wkv, Wo, trace=False):
    nc = build_nc()
    in_maps = make_core_inputs(x, Wq, Wkv, Wo)
    res = run_bass_kernel_spmd(nc, in_maps, core_ids=list(range(NCORES)),
                               trace=trace)
    y = np.zeros((B, T, C), np.float32)
    for c in range(NCORES):
        y[c // 4] += np.asarray(res.results[c]["y"], np.float32)
    if trace:
        kernel.last_exec_time_ns = res.exec_time_ns
        kernel.last_results = res
    return y


# revision 11
# speedup vs baseline: 1.6433x; 1.1415x over previous
"""GroupedQueryAttention TRN2 kernel (bf16).

Sharding: 4-way tensor-parallel over heads x 2-way data-parallel over batch.
Core c handles batch b=c//4 and head-group gc=c%4 (kv heads {2gc, 2gc+1},
q heads (hkv, g) for g in 0..3 -> 8 q heads per core).

Per-core device program (T=2048, C=2048, D=64), all matmul operands bf16
(PSUM accumulation fp32):
  phase A (PE-bound ~82us): Q^T/K^T/V^T projections per 512-col block,
    V PE-transposed into V' [T,130] with ones cols (rowsum trick).
  phase B (Scalar exp-bound): S^T tiles = K_tile Q^T (causal widths),
    exp on ScalarE -> bf16, diag mask on GpSimd, O'^T accumulated on PE
    (row 64 = rowsum), normalize via ones2-matmul broadcast of rowsums +
    DVE reciprocal_approx_fast + DVE multiply.
  phase C (PE-bound): y_tile = O^T.T @ Wo_g, interleaved INTO phase B at
    (j,g) boundaries so PE fills exp-wait stalls with output-proj matmuls.
  Host sums the 4 TP partials per batch in fp32.

PSUM budget in B||C: ps 2x2 banks + po 2x1 + py/pru 2x1 = 8 banks.
"""

import numpy as np
import ml_dtypes

import concourse.bass as bass
import concourse.mybir as mybir
import concourse.tile as tile
from concourse import bacc
from concourse.bass_utils import run_bass_kernel_spmd

H, HKV, D, G = 32, 8, 64, 4
B, T, C = 2, 2048, 2048
P = 128
NCORES = 8
F32 = mybir.dt.float32
BF = mybir.dt.bfloat16
BFNP = ml_dtypes.bfloat16

NT = T // 512   # 4 q blocks of 512
NK = C // P     # 16 contraction tiles
NTT = T // P    # 16 key/row tiles

_NC_CACHE = None


def build_kernel(nc, tc, ins, outs):
    xTr = ins["xT"].rearrange("(ko p) t -> p ko t", p=P)
    y = outs["y"]
    EXP = mybir.ActivationFunctionType.Exp
    CPY = mybir.ActivationFunctionType.Copy

    # ---- persistent SBUF ----
    persist = tc._persist_pool
    qT_sb = persist.tile([P, G, T], BF, name="qT_sb", tag="qT_sb")
    kT_sb = persist.tile([P, T], BF, name="kT_sb", tag="kT_sb")
    v_sb = persist.tile([P, NTT, 130], BF, name="v_sb", tag="v_sb")
    oT_sb = persist.tile([P, G, T], BF, name="oT_sb", tag="oT_sb")
    wo_sb = persist.tile([P, G, C], BF, name="wo_sb", tag="wo_sb")
    tri_sb = persist.tile([P, P], BF, name="tri_sb", tag="tri_sb")
    iden_sb = persist.tile([P, P], BF, name="iden_sb", tag="iden_sb")
    ones2_sb = persist.tile([2, P], BF, name="ones2_sb", tag="ones2_sb")

    nc.sync.dma_start(tri_sb[:], ins["tri"][:])
    nc.sync.dma_start(iden_sb[:], ins["iden"][:])
    nc.sync.dma_start(ones2_sb[:], ins["ones2"][:])
    # ones columns of V' (64 and 129 per key tile); data columns overwritten
    nc.sync.dma_start(v_sb[:], ins["vinit"][:])

    # ================= phase A: projections =================
    with (
        tc.tile_pool(name="wproj", bufs=1) as wpool,
        tc.tile_pool(name="xt", bufs=2) as xpool,
        tc.tile_pool(name="vtt", bufs=2) as vttpool,
        tc.tile_pool(name="pp", bufs=6, space="PSUM") as pp,
        tc.tile_pool(name="pvt", bufs=2, space="PSUM") as pvt,
    ):
        wq_sb = wpool.tile([P, NK, 512], BF, name="wq_sb")
        wk_sb = wpool.tile([P, NK, P], BF, name="wk_sb")
        wv_sb = wpool.tile([P, NK, P], BF, name="wv_sb")
        xt0 = xpool.tile([P, NK, 512], BF, tag="xt", name="xt_0")
        # interleave weight/x chunks so the k=0 matmuls start ~3us in
        wqr = ins["wqT"].rearrange("(ko p) m -> p ko m", p=P)
        for c4 in range(4):
            nc.sync.dma_start(wq_sb[:, 4 * c4:4 * c4 + 4, :],
                              wqr[:, 4 * c4:4 * c4 + 4, :])
            nc.sync.dma_start(xt0[:, 4 * c4:4 * c4 + 4, 0:512],
                              xTr[:, 4 * c4:4 * c4 + 4, 0:512])
            if c4 == 0:
                nc.sync.dma_start(wk_sb[:],
                                  ins["wkT"].rearrange("(ko p) m -> p ko m", p=P))
                nc.sync.dma_start(wv_sb[:],
                                  ins["wvT"].rearrange("(ko p) m -> p ko m", p=P))

        for tb in range(NT):
            ts = slice(tb * 512, (tb + 1) * 512)
            if tb == 0:
                xt = xt0
            else:
                xt = xpool.tile([P, NK, 512], BF, tag="xt", name=f"xt_{tb}")
                nc.sync.dma_start(xt[:], xTr[:, :, ts])
            psq = [pp.tile([P, 512], F32, tag="pp", name=f"psq_{tb}_{g}")
                   for g in range(G)]
            psk = pp.tile([P, 512], F32, tag="pp", name=f"psk_{tb}")
            psv = pp.tile([P, 512], F32, tag="pp", name=f"psv_{tb}")
            for k in range(NK):
                st, sp = (k == 0), (k == NK - 1)
                for g in range(G):
                    nc.tensor.matmul(psq[g][:],
                                     wq_sb[:, k, g * P:(g + 1) * P],
                                     xt[:, k, :], start=st, stop=sp)
                nc.tensor.matmul(psk[:], wk_sb[:, k, :], xt[:, k, :],
                                 start=st, stop=sp)
                nc.tensor.matmul(psv[:], wv_sb[:, k, :], xt[:, k, :],
                                 start=st, stop=sp)
            vtt = vttpool.tile([P, 512], BF, tag="vtt", name=f"vtt_{tb}")
            # copies split Scalar/DVE to shorten the per-tb tail
            nc.scalar.activation(qT_sb[:, 0, ts], psq[0][:], CPY)
            nc.scalar.activation(qT_sb[:, 1, ts], psq[1][:], CPY)
            nc.scalar.activation(kT_sb[:, ts], psk[:], CPY)
            nc.vector.tensor_copy(qT_sb[:, 2, ts], psq[2][:])
            nc.vector.tensor_copy(qT_sb[:, 3, ts], psq[3][:])
            nc.vector.tensor_copy(vtt[:], psv[:])
            # V^T -> V (PE transpose) into V' layout with ones cols
            for dd in range(4):
                tt = 4 * tb + dd
                pt_ = pvt.tile([P, P], BF, tag="pvt", name=f"pvt_{tt}")
                nc.tensor.transpose(pt_[:], vtt[:, dd * P:(dd + 1) * P],
                                    iden_sb[:])
                nc.vector.tensor_copy(v_sb[:, tt, 0:64], pt_[:, 0:64])
                nc.vector.tensor_copy(v_sb[:, tt, 65:129], pt_[:, 64:128])

    # ================= phase B || C =================
    for c4 in range(4):
        nc.sync.dma_start(
            wo_sb[:, c4, :],
            ins["woR"][128 * c4:128 * (c4 + 1), :].rearrange(
                "(m p) c -> p m c", p=P))

    with (
        tc.tile_pool(name="ps", bufs=2, space="PSUM") as pspool,
        tc.tile_pool(name="po", bufs=2, space="PSUM") as popool,
        tc.tile_pool(name="py", bufs=2, space="PSUM") as pypool,
        tc.tile_pool(name="ptp", bufs=6) as ptpool,
        tc.tile_pool(name="rs", bufs=4) as rspool,
        tc.tile_pool(name="rb", bufs=2) as rbpool,
        tc.tile_pool(name="ysb", bufs=2) as ypool,
    ):
        def emit_B_tiles(j, g):
            """Scores + exp + mask + PV accumulation; returns po + rowsum
            copies (issued immediately so DVE overlaps the following C
            chunk)."""
            qs0 = j * 512
            po = [popool.tile([P, 512], F32, tag="po", name=f"po_{j}_{g}_{h}")
                  for h in range(2)]
            ilast = 4 * j + 3
            for i in range(4 * j + 4):
                loc = max(0, P * i - qs0)
                ps = pspool.tile([P, 2, 512], F32, tag="ps",
                                 name=f"ps_{j}_{g}_{i}")
                pt_ = ptpool.tile([P, 2, 512], BF, tag="pt",
                                  name=f"pt_{j}_{g}_{i}")
                for h in range(2):
                    nc.tensor.matmul(
                        ps[:, h, loc:512],
                        kT_sb[h * 64:(h + 1) * 64, i * P:(i + 1) * P],
                        qT_sb[h * 64:(h + 1) * 64, g, qs0 + loc:qs0 + 512],
                        start=True, stop=True,
                    )
                nc.scalar.activation(pt_[:, :, loc:512], ps[:, :, loc:512],
                                     EXP, scale=0.125)
                if i >= 4 * j:  # diagonal tile: mask strict-lower triangle
                    nc.vector.tensor_mul(
                        pt_[:, :, loc:loc + P], pt_[:, :, loc:loc + P],
                        tri_sb[:, None, :].to_broadcast([P, 2, P]),
                    )
                for h in range(2):
                    nc.tensor.matmul(
                        po[h][0:65, loc:512],
                        v_sb[:, i, h * 65:h * 65 + 65],
                        pt_[:, h, loc:512],
                        start=(i == 0), stop=(i == ilast),
                    )
            rs = [rspool.tile([1, 512], BF, tag="rs", name=f"rs_{j}_{g}_{h}")
                  for h in range(2)]
            for h in range(2):
                nc.vector.tensor_copy(rs[h][:], po[h][64:65, :])
            return po, rs

        def emit_B_norm(j, g, po, rs):
            """Broadcast rowsums (PE), reciprocal + normalize (DVE).
            Emitted after the interleaved C chunk so PE never waits on the
            DVE rowsum copies."""
            qs0 = j * 512
            pru = pypool.tile([P, 512], F32, tag="py", name=f"pru_{j}_{g}")
            for h in range(2):
                nc.tensor.matmul(pru[h * 64:(h + 1) * 64, :], ones2_sb[0:1, 0:64],
                                 rs[h][:], start=True, stop=True)
            rb = rbpool.tile([P, 512], F32, tag="rb", name=f"rb_{j}_{g}")
            nc.vector.reciprocal_approx_fast(rb[:], pru[:])
            for h in range(2):
                nc.vector.tensor_mul(
                    oT_sb[h * 64:(h + 1) * 64, g, qs0:qs0 + 512],
                    po[h][0:64, :], rb[h * 64:(h + 1) * 64, :],
                )

        def emit_C_tt(tt):
            ysb = ypool.tile([P, T], BF, tag="ysb", name=f"y_{tt}")
            for cb in range(NT):
                py = pypool.tile([P, 512], F32, tag="py", name=f"py_{tt}_{cb}")
                for m in range(G):
                    nc.tensor.matmul(
                        py[:], oT_sb[:, m, tt * P:(tt + 1) * P],
                        wo_sb[:, m, cb * 512:(cb + 1) * 512],
                        start=(m == 0), stop=(m == G - 1),
                    )
                nc.vector.tensor_copy(ysb[:, cb * 512:(cb + 1) * 512], py[:])
            nc.sync.dma_start(y[tt * P:(tt + 1) * P, :], ysb[:])

        for j in range(NT):
            for g in range(G):
                po, rs = emit_B_tiles(j, g)
                if j >= 1:
                    emit_C_tt(4 * (j - 1) + g)
                emit_B_norm(j, g, po, rs)
        for dd in range(4):
            emit_C_tt(12 + dd)


def build_nc():
    global _NC_CACHE
    if _NC_CACHE is not None:
        return _NC_CACHE
    nc = bacc.Bacc("TRN2", debug=False, target_bir_lowering=False,
                   num_devices=NCORES)
    ins = {
        "xT": nc.dram_tensor("xT", [C, T], BF, kind="ExternalInput").ap(),
        "wqT": nc.dram_tensor("wqT", [C, 512], BF, kind="ExternalInput").ap(),
        "wkT": nc.dram_tensor("wkT", [C, P], BF, kind="ExternalInput").ap(),
        "wvT": nc.dram_tensor("wvT", [C, P], BF, kind="ExternalInput").ap(),
        "woR": nc.dram_tensor("woR", [512, C], BF, kind="ExternalInput").ap(),
        "tri": nc.dram_tensor("tri", [P, P], BF, kind="ExternalInput").ap(),
        "iden": nc.dram_tensor("iden", [P, P], BF, kind="ExternalInput").ap(),
        "ones2": nc.dram_tensor("ones2", [2, P], BF, kind="ExternalInput").ap(),
        "vinit": nc.dram_tensor("vinit", [P, NTT, 130], BF,
                                kind="ExternalInput").ap(),
    }
    outs = {"y": nc.dram_tensor("y", [T, C], BF, kind="ExternalOutput").ap()}
    with tile.TileContext(nc) as tc:
        with tc.tile_pool(name="persist", bufs=1) as persist:
            tc._persist_pool = persist
            build_kernel(nc, tc, ins, outs)
    nc.compile()
    _NC_CACHE = nc
    return nc


def make_core_inputs(x, Wq, Wkv, Wo):
    """Host-side shard + pre-transpose + bf16 cast. Returns 8 in_maps."""
    x = np.asarray(x, np.float32)
    Wq = np.asarray(Wq, np.float32)
    Wkv = np.asarray(Wkv, np.float32)
    Wo = np.asarray(Wo, np.float32)
    tri = np.triu(np.ones((P, P), np.float32))   # keep t_local >= s_local
    iden = np.eye(P, dtype=np.float32)
    # ones2: partitions 0-63 of the broadcast get rowsum h0, 64-127 get h1
    ones2 = np.zeros((2, P), np.float32)
    ones2[0, 0:64] = 1.0
    ones2[1, 64:128] = 1.0
    in_maps = []
    for c in range(NCORES):
        b, gc = c // 4, c % 4
        xT = np.ascontiguousarray(x[b].T)                       # [C, T]
        Wq4 = Wq.reshape(HKV, G, D, C)[2 * gc:2 * gc + 2]       # [2, G, D, C]
        wqT = np.ascontiguousarray(
            np.transpose(Wq4, (1, 0, 2, 3)).reshape(512, C).T)  # [C, (g,kv,d)]
        wkT = np.ascontiguousarray(Wkv[2 * gc * 64:(2 * gc + 2) * 64].T)
        wvT = np.ascontiguousarray(
            Wkv[HKV * D + 2 * gc * 64:HKV * D + (2 * gc + 2) * 64].T)
        Wo4 = Wo.reshape(C, HKV, G, D)[:, 2 * gc:2 * gc + 2]    # [C, 2, G, D]
        woR = np.ascontiguousarray(
            np.transpose(Wo4, (2, 1, 3, 0)).reshape(512, C))    # [(g,kv,d), C]
        in_maps.append({
            "xT": xT.astype(BFNP), "wqT": wqT.astype(BFNP),
            "wkT": wkT.astype(BFNP), "wvT": wvT.astype(BFNP),
            "woR": woR.astype(BFNP), "tri": tri.astype(BFNP),
            "iden": iden.astype(BFNP), "ones2": ones2.astype(BFNP),
            "vinit": np.ones((P, NTT, 130), BFNP),
        })
    return in_maps


def kernel(x, Wq, Wkv, Wo, trace=False):
    nc = build_nc()
    in_maps = make_core_inputs(x, Wq, Wkv, Wo)
    res = run_bass_kernel_spmd(nc, in_maps, core_ids=list(range(NCORES)),
                               trace=trace)
    y = np.zeros((B, T, C), np.float32)
    for c in range(NCORES):
        y[c // 4] += np.asarray(res.results[c]["y"], np.float32)
    if trace:
        kernel.last_exec_time_ns = res.exec_time_ns
        kernel.last_results = res
    return y
